# revision 14
# baseline (speedup 1.0000x reference)
"""CPAMDec attention-decoder kernel for 8 Trainium2 NeuronCores.

Reference computation (per batch n of N=8):
    q  = x_n^T @ wq^T + bq          (HW=4096, C4=128)
    k  = y_n @ wk^T + bk            (K=32, C4=128)
    v  = y_n @ wv^T + bv            (K=32, C=512)
    attn = softmax(q @ k^T, axis=-1)        (HW, K)
    out = scale * (v^T @ attn^T) + x_n      (C, HW)

Sharding: pure data parallel - core i computes batch i; params replicated.

Key optimizations:
  - bf16 I/O. x and out move over HBM as bf16 (8MB -> 4MB each way per
    core); HBM-per-NC (~358 GB/s) is the binding roofline. rel-err of the
    full bf16 pipeline is ~3e-3, well under the 2e-2 gate.
  - wq folding: e[j,p] = sum_c EM[c,j] x[c,p] with EM = wq^T @ (k^T+bk)
    computed once in the prologue. The per-chunk q stage (4 matmuls + an
    ACT copy) disappears; energy comes straight from x.
  - bq contributes a per-key bias e_b[j] = sum_o bq[o]*ktb[o,j], applied
    inside the exp() activation (exact algebra).
  - bv enters as a per-partition scalar in the fused output STT
    osb = (o_ps + s*bv[c]) + x, using sum_j attn[p,j] = 1. The 4 STTs
    per chunk are split 2 on DVE / 2 on GpSimd(Pool).
  - Consts ride in 3 packed DRAM params (DMA issue costs ~0.7us of queue
    time each; 10 separate loads would serialize startup by ~7us).
  - PE warm-up dummies ramp the HAM clock gate while DMAs land.
"""

import sys

sys.path.insert(0, "/opt/trn_rl_repo")

import numpy as np
import ml_dtypes

import concourse.bacc as bacc
import concourse.mybir as mybir
import concourse.tile as tile
from concourse.alu_op_type import AluOpType
from concourse.bass_utils import run_bass_kernel_spmd

F32 = mybir.dt.float32
BF16 = mybir.dt.bfloat16
AF = mybir.ActivationFunctionType
BF = ml_dtypes.bfloat16

N, C, H, W, K = 8, 512, 64, 64, 32
HW = H * W            # 4096
C4 = C // 4           # 128
PC = 512              # free-dim chunk (1 PSUM bank of fp32)
NPC = HW // PC        # 8 chunks
KC = C // 128         # 4 contraction chunks
CT = C // 128         # 4 output row-tiles


def _emit(nc, tc):
    sync = nc.sync
    cdma = nc.sync        # consts share the SP ring: FIFO order beats x

    with (
        tc.tile_pool(name="const", bufs=1) as cst,
        tc.tile_pool(name="xbuf", bufs=1) as xp,
        tc.tile_pool(name="work", bufs=3) as wk_pool,
        tc.tile_pool(name="ps", bufs=2, space="PSUM") as ps,
    ):
        # ---------------- constant loads (3 packed DMAs) ----------------
        # pa = wqo[128,512] | bqb[128,32]            bf16
        # pf = bk[128,1] | bvt[128,4]                f32
        # pb = wkt | yt | wvt  as [512, 768] -> [128, 4, 768]  bf16
        pb1 = cst.tile([128, KC, 256], BF16, name="pb1", tag="pb1")
        cdma.dma_start(pb1[:],
                       nc.t.pb1[:].rearrange("(k p) f -> p k f", p=128))
        pa = cst.tile([128, C + K + C], BF16, name="pa", tag="pa")
        cdma.dma_start(pa[:], nc.t.pa[:])
        wqo = pa[:, 0:C]
        bqb = pa[:, C:C + K]
        bvb32 = pa[0:K, C + K:C + K + C]
        pf = cst.tile([128, 1], F32, name="pf", tag="pf")
        cdma.dma_start(pf[:], nc.t.pf[:])
        bk_sb = pf[:, 0:1]
        pb2 = cst.tile([128, KC, C], BF16, name="pb2", tag="pb2")

        def wkt(k):
            return pb1[:, k, 0:C4]

        def yt(k):
            return pb1[:, k, 128:256]

        def wvt(k):
            return pb2[:, k, :]

        ones32 = cst.tile([K, 128], BF16, name="ones32", tag="ones32")
        nc.gpsimd.memset(ones32[:], 1.0)

        # x chunk-pair loads (1MB each, contiguous per partition) on the
        # same SP ring, FIFO behind the consts. SBUF keeps all 8 chunks.
        xs = [None] * NPC

        def load_pair(j):
            t = xp.tile([128, 2, KC, PC], BF16, name=f"xp{j}", tag=f"xp{j}")
            src = nc.t.x[:, j * 2 * KC * PC:(j + 1) * 2 * KC * PC].rearrange(
                "p (c k f) -> p c k f", c=2, k=KC)
            sync.dma_start(t[:], src)
            xs[2 * j] = t[:, 0]
            xs[2 * j + 1] = t[:, 1]

        load_pair(0)
        cdma.dma_start(pb2[:],
                       nc.t.pb2[:].rearrange("(k p) f -> p k f", p=128))
        for j in range(1, 4):
            load_pair(j)

        # ---------------- PE warm-up ----------------
        # Ramp the HAM clock gate (1.2 -> 2.4 GHz after ~3.4us sustained)
        # while DMAs land. Reads pa (first const to arrive).
        dmy_ps = ps.tile([128, PC], F32, name="dmy_ps", tag="s", bufs=2)
        for _ in range(6):
            nc.tensor.matmul(dmy_ps[:], pb1[:, 0, 0:128], pb1[:, 0:2, :],
                             start=True, stop=True)

        # Load the exp ACT table before steady state (Copy/Identity live in
        # every table, so this is the only table load).
        acttbl = cst.tile([128, 8], BF16, name="acttbl", tag="acttbl")
        nc.scalar.activation(out=acttbl[:], in_=pb1[:, 0, 0:8], func=AF.Exp,
                             bias=0.0, scale=1.0)

        # ---------------- prologue ----------------
        # ktb[o,j] = sum_c wk[o,c] y[j,c] + bk[o]   (4K=128 j-replicas)
        kt_ps = ps.tile([C4, 4 * K], F32, name="kt_ps", tag="e", bufs=2)
        for k in range(KC):
            nc.tensor.matmul(kt_ps[:], wkt(k), yt(k),
                             start=(k == 0), stop=(k == KC - 1))
        ktb4 = cst.tile([C4, 4 * K], BF16, name="ktb4", tag="ktb4")
        nc.scalar.activation(out=ktb4[:], in_=kt_ps[:], func=AF.Identity,
                             bias=bk_sb, scale=1.0)

        # EM[c,j] = sum_o wq[o,c] ktb[o,j]  (c-tiled: [128, KC, 128])
        em_ps = ps.tile([128, KC, 128], F32, name="em_ps", tag="e", bufs=2)
        for k in range(KC):
            nc.tensor.matmul(em_ps[:, k, :], wqo[:, k * 128:(k + 1) * 128],
                             ktb4[:], start=True, stop=True)
        em_sb = cst.tile([128, KC, 128], BF16, name="em_sb", tag="em_sb")
        nc.scalar.activation(out=em_sb[:], in_=em_ps[:], func=AF.Copy,
                             bias=0.0, scale=1.0)

        # v[j,c] = sum_cl y[j,cl] wv(scaled)[c,cl] + s*bv[c]
        # (emitted after e(0) so the first energy chunk leads the PE queue;
        # wv arrives behind the first x pair anyway)
        vstack = cst.tile([128, 128], BF16, name="vstack", tag="vstack")

        def emit_v():
            v_ps = ps.tile([K, C], F32, name="v_ps", tag="s", bufs=2)
            for k in range(KC):
                nc.tensor.matmul(v_ps[:], yt(k)[:, 0:K], wvt(k),
                                 start=(k == 0), stop=(k == KC - 1))
            v_sb = cst.tile([K, C], BF16, name="v_sb", tag="v_sb")
            nc.vector.tensor_tensor(v_sb[:], v_ps[:], bvb32,
                                    op=AluOpType.add)
            # vstack[32*ct + j, m] = v_sb[j, 128*ct + m]
            for ct in range(CT):
                nc.gpsimd.dma_start(
                    vstack[32 * ct:32 * (ct + 1), :],
                    v_sb[:, 128 * ct:128 * (ct + 1)])

        # e_b[j] = sum_o bq[o] ktb[o,j] -> exp bias, per partition
        eb_ps = ps.tile([4 * K, K], F32, name="eb_ps", tag="o", bufs=2)
        nc.tensor.matmul(eb_ps[:], ktb4[:], bqb[:], start=True, stop=True)
        e_b4 = cst.tile([4 * K, 1], F32, name="e_b4", tag="e_b4")
        nc.scalar.activation(out=e_b4[:], in_=eb_ps[:, 0:1], func=AF.Copy,
                             bias=0.0, scale=1.0)

        # ------------- software-pipelined main loop over column chunks ----
        #   step i:  e/exp(i)   sum/rec/mul(i-1)   out-mm/stt/store(i-2)
        expts = [None] * NPC
        attns = [None] * NPC

        def stage_e(pc):
            e_ps = ps.tile([128, PC], F32, name=f"e_ps{pc}", tag="e", bufs=2)
            for k in range(KC):
                nc.tensor.matmul(e_ps[:], em_sb[:, k, :], xs[pc][:, k, :],
                                 start=(k == 0), stop=(k == KC - 1))
            expt = wk_pool.tile([128, PC], BF16, name="expt", tag="expt",
                                bufs=3)
            nc.scalar.activation(out=expt[:], in_=e_ps[:], func=AF.Exp,
                                 bias=e_b4[:], scale=1.0)
            expts[pc] = expt

        def stage_s(pc):
            s_ps = ps.tile([128, PC], F32, name=f"s_ps{pc}", tag="s", bufs=2)
            nc.tensor.matmul(s_ps[:], ones32[:], expts[pc][0:K, :],
                             start=True, stop=True)
            rec = wk_pool.tile([128, PC], F32, name="rec", tag="rec", bufs=2)
            nc.vector.reciprocal_approx_fast(out=rec[:], in_=s_ps[:])
            attn = wk_pool.tile([128, PC], BF16, name="attn", tag="attn",
                                bufs=3)
            nc.gpsimd.tensor_tensor(attn[:], expts[pc][:], rec[:],
                                    op=AluOpType.mult)
            attns[pc] = attn

        def stage_out(pc):
            xt = xs[pc]
            attn = attns[pc]
            osb = wk_pool.tile([128, CT, PC], BF16, name="osb", tag="osb",
                               bufs=3)
            # two double-bank PSUM tiles; one 1024-col DVE add per pair
            for h in range(2):
                o_ps = ps.tile([128, 2, PC], F32, name=f"o_ps{pc}_{h}",
                               tag="o", bufs=2)
                for i in range(2):
                    ct = 2 * h + i
                    nc.tensor.matmul(o_ps[:, i, :],
                                     vstack[32 * ct:32 * (ct + 1), :],
                                     attn[32 * ct:32 * (ct + 1), :],
                                     start=True, stop=True,
                                     tile_position=(32 * ct, 0))
                nc.vector.tensor_tensor(osb[:, 2 * h:2 * h + 2, :], o_ps[:],
                                        xt[:, 2 * h:2 * h + 2, :],
                                        op=AluOpType.add)
            dst = nc.t.out[:, pc * 2048:(pc + 1) * 2048].rearrange(
                "p (k f) -> p k f", k=CT)
            sync.dma_start(dst, osb[:])

        for step in range(NPC + 3):
            if step < NPC:
                stage_e(step)
            if step == 0:
                emit_v()
            if 0 <= step - 1 < NPC:
                stage_s(step - 1)
            if 0 <= step - 2 < NPC:
                stage_out(step - 2)


class _T:
    """Attribute access to declared dram params."""
    def __init__(self):
        self.__dict__ = {}


_NC_CACHE = []


def _build():
    if _NC_CACHE:
        return _NC_CACHE[0]
    nc = bacc.Bacc(target_bir_lowering=False)
    nc.t = _T()
    t = nc.t
    t.x = nc.declare_dram_parameter("x", [128, NPC * KC * PC], BF16,
                                    isOutput=False)
    t.pa = nc.declare_dram_parameter("pa", [128, C + K + C], BF16,
                                     isOutput=False)
    t.pf = nc.declare_dram_parameter("pf", [128, 1], F32, isOutput=False)
    t.pb1 = nc.declare_dram_parameter("pb1", [C, 256], BF16, isOutput=False)
    t.pb2 = nc.declare_dram_parameter("pb2", [C, C], BF16, isOutput=False)
    t.out = nc.declare_dram_parameter("out", [128, NPC * KC * PC], BF16,
                                      isOutput=True)
    with tile.TileContext(nc) as tc:
        _emit(nc, tc)
    nc.finalize()
    _NC_CACHE.append(nc)
    return nc


def _in_maps(x, y, wq, bq, wk, bk, wv, bv, scale):
    x = np.ascontiguousarray(x, dtype=np.float32).reshape(N, C, HW).astype(BF)
    # xsw[p, pc, k, f] = x[k*128+p, pc*512+f] -> contiguous per-chunk loads
    x = np.ascontiguousarray(
        x.reshape(N, KC, 128, NPC, PC).transpose(0, 2, 3, 1, 4).reshape(
            N, 128, NPC * KC * PC))
    yt = np.ascontiguousarray(
        np.tile(np.transpose(y, (0, 2, 1)), (1, 1, 4))).astype(BF)
    s = float(np.float32(scale).reshape(-1)[0])
    wqo = np.ascontiguousarray(wq, dtype=np.float32).astype(BF)
    wkt = np.ascontiguousarray(wk.T, dtype=np.float32).astype(BF)
    wvt = np.ascontiguousarray(wv.T * s, dtype=np.float32).astype(BF)
    bqb = np.ascontiguousarray(
        np.broadcast_to(np.float32(bq).reshape(C4, 1), (C4, K))).astype(BF)
    bvb = np.zeros((128, C), dtype=BF)
    bvb[0:K, :] = np.float32(bv).reshape(1, C) * s
    bkc = np.ascontiguousarray(bk, dtype=np.float32).reshape(C4, 1)
    pa = np.concatenate([wqo, bqb, bvb], axis=1)            # [128, 1056]
    return [
        {
            "x": x[i],
            "pa": pa,
            "pf": bkc,
            "pb1": np.concatenate([wkt, yt[i]], axis=1),
            "pb2": wvt,
        }
        for i in range(N)
    ]


def _run(inputs, **kwargs):
    nc = _build()
    return run_bass_kernel_spmd(nc, _in_maps(**inputs),
                                core_ids=list(range(N)), **kwargs)


def kernel(**inputs) -> np.ndarray:
    res = _run(inputs)
    out = np.stack([np.asarray(res.results[i]["out"], dtype=np.float32)
                    for i in range(N)])
    # oswz[p, pc, ct, f] = out[ct*128+p, pc*512+f]
    out = out.reshape(N, 128, NPC, CT, PC).transpose(0, 3, 1, 2, 4)
    return np.ascontiguousarray(out).reshape(N, C, H, W)


# revision 15
# speedup vs baseline: 1.1346x; 1.1346x over previous
"""CPAMDec attention-decoder kernel for 8 Trainium2 NeuronCores.

Reference computation (per batch n of N=8):
    q  = x_n^T @ wq^T + bq          (HW=4096, C4=128)
    k  = y_n @ wk^T + bk            (K=32, C4=128)
    v  = y_n @ wv^T + bv            (K=32, C=512)
    attn = softmax(q @ k^T, axis=-1)        (HW, K)
    out = scale * (v^T @ attn^T) + x_n      (C, HW)

Sharding: pure data parallel - core i computes batch i; params replicated.

Key optimizations:
  - bf16 I/O. x and out move over HBM as bf16 (8MB -> 4MB each way per
    core); HBM-per-NC (~358 GB/s) is the binding roofline. rel-err of the
    full bf16 pipeline is ~3e-3, well under the 2e-2 gate.
  - wq folding: e[j,p] = sum_c EM[c,j] x[c,p] with EM = wq^T @ (k^T+bk)
    computed once in the prologue. The per-chunk q stage (4 matmuls + an
    ACT copy) disappears; energy comes straight from x.
  - bq contributes a per-key bias e_b[j] = sum_o bq[o]*ktb[o,j], applied
    inside the exp() activation (exact algebra).
  - scale folded into wv on host; s*bv baked into every v row in the
    prologue (sum_j attn = 1), so the per-chunk final is a plain
    o_ps + x tensor_tensor on DVE over double-bank PSUM tiles.
  - Consts ride in 3 packed DRAM params (DMA issue costs ~0.7us of queue
    time each; 10 separate loads would serialize startup by ~7us).
  - PE warm-up dummies ramp the HAM clock gate while DMAs land.
"""

import sys

sys.path.insert(0, "/opt/trn_rl_repo")

import numpy as np
import ml_dtypes

import concourse.bacc as bacc
import concourse.mybir as mybir
import concourse.tile as tile
from concourse.alu_op_type import AluOpType
from concourse.bass_utils import run_bass_kernel_spmd

F32 = mybir.dt.float32
BF16 = mybir.dt.bfloat16
AF = mybir.ActivationFunctionType
BF = ml_dtypes.bfloat16

N, C, H, W, K = 8, 512, 64, 64, 32
HW = H * W            # 4096
C4 = C // 4           # 128
PC = 512              # free-dim chunk (1 PSUM bank of fp32)
NPC = HW // PC        # 8 chunks
KC = C // 128         # 4 contraction chunks
CT = C // 128         # 4 output row-tiles


def _emit(nc, tc):
    sync = nc.sync
    cdma = nc.scalar      # consts ride the ACT HWDGE ring

    with (
        tc.tile_pool(name="const", bufs=1) as cst,
        tc.tile_pool(name="xbuf", bufs=1) as xp,
        tc.tile_pool(name="work", bufs=3) as wk_pool,
        tc.tile_pool(name="ps", bufs=2, space="PSUM") as ps,
    ):
        # ---------------- constant loads (3 packed DMAs) ----------------
        # pa = wqo[128,512] | bqb[128,32] | bvb[128,512]   bf16
        # pf = bk[128,1]                                   f32
        # pb = wkt | yt | wvt  as [512, 768] -> [128, 4, 768]  bf16
        pa = cst.tile([128, C + K + C], BF16, name="pa", tag="pa")
        cdma.dma_start(pa[:], nc.t.pa[:])
        wqo = pa[:, 0:C]
        bqb = pa[:, C:C + K]
        bvb32 = pa[0:K, C + K:C + K + C]
        pf = cst.tile([128, 1], F32, name="pf", tag="pf")
        cdma.dma_start(pf[:], nc.t.pf[:])
        bk_sb = pf[:, 0:1]
        pb = cst.tile([128, KC, 128 + 128 + C], BF16, name="pb", tag="pb")
        cdma.dma_start(pb[:], nc.t.pb[:].rearrange("(k p) f -> p k f", p=128))

        def wkt(k):
            return pb[:, k, 0:C4]

        def yt(k):
            return pb[:, k, 128:128 + 4 * K]

        def wvt(k):
            return pb[:, k, 256:256 + C]

        ones32 = cst.tile([K, 128], BF16, name="ones32", tag="ones32")
        nc.gpsimd.memset(ones32[:], 1.0)

        # x column chunks: (128 part, 4 c-tiles, PC cols) strided loads on
        # the SP ring. SBUF is plentiful: keep all 8 resident.
        xs = [None] * NPC

        def load_chunk(pc):
            t = xp.tile([128, KC, PC], BF16, name=f"xs{pc}", tag=f"xs{pc}")
            src = nc.t.x[:, pc * PC:(pc + 1) * PC].rearrange(
                "(k p) f -> p k f", p=128)
            sync.dma_start(t[:], src)
            xs[pc] = t

        for pc in range(4):
            load_chunk(pc)

        # ---------------- PE warm-up ----------------
        # Ramp the HAM clock gate (1.2 -> 2.4 GHz after ~3.4us sustained)
        # while DMAs land. Reads pa (first const to arrive).
        dmy_ps = ps.tile([128, PC], F32, name="dmy_ps", tag="s", bufs=1)
        for _ in range(7):
            nc.tensor.matmul(dmy_ps[:], pa[:, 0:128], wqo[:],
                             start=True, stop=True)

        # Load the exp ACT table before steady state (Copy/Identity live in
        # every table, so this is the only table load).
        acttbl = cst.tile([128, 8], BF16, name="acttbl", tag="acttbl")
        nc.scalar.activation(out=acttbl[:], in_=pa[:, 0:8], func=AF.Exp,
                             bias=0.0, scale=1.0)

        # ---------------- prologue ----------------
        # ktb[o,j] = sum_c wk[o,c] y[j,c] + bk[o]   (4K=128 j-replicas)
        kt_ps = ps.tile([C4, 4 * K], F32, name="kt_ps", tag="e", bufs=2)
        for k in range(KC):
            nc.tensor.matmul(kt_ps[:], wkt(k), yt(k),
                             start=(k == 0), stop=(k == KC - 1))
        ktb4 = cst.tile([C4, 4 * K], BF16, name="ktb4", tag="ktb4")
        nc.scalar.activation(out=ktb4[:], in_=kt_ps[:], func=AF.Identity,
                             bias=bk_sb, scale=1.0)

        # EM[c,j] = sum_o wq[o,c] ktb[o,j]  (c-tiled: [128, KC, 128])
        em_ps = ps.tile([128, KC, 128], F32, name="em_ps", tag="e", bufs=2)
        for k in range(KC):
            nc.tensor.matmul(em_ps[:, k, :], wqo[:, k * 128:(k + 1) * 128],
                             ktb4[:], start=True, stop=True)
        em_sb = cst.tile([128, KC, 128], BF16, name="em_sb", tag="em_sb")
        nc.scalar.activation(out=em_sb[:], in_=em_ps[:], func=AF.Copy,
                             bias=0.0, scale=1.0)

        # v[j,c] = sum_cl y[j,cl] wv(scaled)[c,cl], then += s*bv[c]
        v_ps = ps.tile([K, C], F32, name="v_ps", tag="s", bufs=1)
        for k in range(KC):
            nc.tensor.matmul(v_ps[:], yt(k)[:, 0:K], wvt(k),
                             start=(k == 0), stop=(k == KC - 1))
        v_sb = cst.tile([K, C], BF16, name="v_sb", tag="v_sb")
        nc.vector.tensor_tensor(v_sb[:], v_ps[:], bvb32,
                                op=AluOpType.add)
        # partition-stacked copy for row-packed final matmuls:
        # vstack[32*ct + j, m] = v_sb[j, 128*ct + m]
        vstack = cst.tile([128, 128], BF16, name="vstack", tag="vstack")
        for ct in range(CT):
            nc.gpsimd.dma_start(
                vstack[32 * ct:32 * (ct + 1), :],
                v_sb[:, 128 * ct:128 * (ct + 1)])

        # e_b[j] = sum_o bq[o] ktb[o,j] -> exp bias, per partition
        eb_ps = ps.tile([4 * K, K], F32, name="eb_ps", tag="o", bufs=2)
        nc.tensor.matmul(eb_ps[:], ktb4[:], bqb[:], start=True, stop=True)
        e_b4 = cst.tile([4 * K, 1], F32, name="e_b4", tag="e_b4")
        nc.scalar.activation(out=e_b4[:], in_=eb_ps[:, 0:1], func=AF.Copy,
                             bias=0.0, scale=1.0)

        # ------------- software-pipelined main loop over column chunks ----
        #   step i:  e/exp(i)   sum/rec/mul(i-1)   out-mm/add/store(i-2)
        expts = [None] * NPC
        attns = [None] * NPC

        def stage_e(pc):
            e_ps = ps.tile([128, PC], F32, name=f"e_ps{pc}", tag="e", bufs=2)
            for k in range(KC):
                nc.tensor.matmul(e_ps[:], em_sb[:, k, :], xs[pc][:, k, :],
                                 start=(k == 0), stop=(k == KC - 1))
            expt = wk_pool.tile([128, PC], BF16, name="expt", tag="expt",
                                bufs=3)
            nc.scalar.activation(out=expt[:], in_=e_ps[:], func=AF.Exp,
                                 bias=e_b4[:], scale=1.0)
            expts[pc] = expt

        def stage_s(pc):
            s_ps = ps.tile([128, PC], F32, name=f"s_ps{pc}", tag="s", bufs=1)
            nc.tensor.matmul(s_ps[:], ones32[:], expts[pc][0:K, :],
                             start=True, stop=True)
            rec = wk_pool.tile([128, PC], F32, name="rec", tag="rec", bufs=2)
            nc.vector.reciprocal_approx_fast(out=rec[:], in_=s_ps[:])
            attn = wk_pool.tile([128, PC], BF16, name="attn", tag="attn",
                                bufs=3)
            nc.gpsimd.tensor_tensor(attn[:], expts[pc][:], rec[:],
                                    op=AluOpType.mult)
            attns[pc] = attn

        def stage_out(pc):
            xt = xs[pc]
            attn = attns[pc]
            osb = wk_pool.tile([128, CT, PC], BF16, name="osb", tag="osb",
                               bufs=3)
            # two double-bank PSUM tiles; one 1024-col DVE add per pair
            for h in range(2):
                o_ps = ps.tile([128, 2, PC], F32, name=f"o_ps{pc}_{h}",
                               tag="o", bufs=2)
                for i in range(2):
                    ct = 2 * h + i
                    nc.tensor.matmul(o_ps[:, i, :],
                                     vstack[32 * ct:32 * (ct + 1), :],
                                     attn[32 * ct:32 * (ct + 1), :],
                                     start=True, stop=True,
                                     tile_position=(32 * ct, 0))
                nc.vector.tensor_tensor(osb[:, 2 * h:2 * h + 2, :], o_ps[:],
                                        xt[:, 2 * h:2 * h + 2, :],
                                        op=AluOpType.add)
                dst = nc.t.out[2 * h * 128:(2 * h + 2) * 128,
                               pc * PC:(pc + 1) * PC].rearrange(
                    "(k p) f -> p k f", p=128)
                sync.dma_start(dst, osb[:, 2 * h:2 * h + 2, :])

        for step in range(NPC + 3):
            if 1 <= step and step + 3 < NPC:
                load_chunk(step + 3)
            if step < NPC:
                stage_e(step)
            if 0 <= step - 1 < NPC:
                stage_s(step - 1)
            if 0 <= step - 2 < NPC:
                stage_out(step - 2)


class _T:
    """Attribute access to declared dram params."""
    def __init__(self):
        self.__dict__ = {}


_NC_CACHE = []


def _build():
    if _NC_CACHE:
        return _NC_CACHE[0]
    nc = bacc.Bacc(target_bir_lowering=False)
    nc.t = _T()
    t = nc.t
    t.x = nc.declare_dram_parameter("x", [C, HW], BF16, isOutput=False)
    t.pa = nc.declare_dram_parameter("pa", [128, C + K + C], BF16,
                                     isOutput=False)
    t.pf = nc.declare_dram_parameter("pf", [128, 1], F32, isOutput=False)
    t.pb = nc.declare_dram_parameter("pb", [C, 256 + C], BF16, isOutput=False)
    t.out = nc.declare_dram_parameter("out", [C, HW], BF16, isOutput=True)
    with tile.TileContext(nc) as tc:
        _emit(nc, tc)
    nc.finalize()
    _NC_CACHE.append(nc)
    return nc


def _in_maps(x, y, wq, bq, wk, bk, wv, bv, scale):
    x = np.ascontiguousarray(x, dtype=np.float32).reshape(N, C, HW).astype(BF)
    yt = np.ascontiguousarray(
        np.tile(np.transpose(y, (0, 2, 1)), (1, 1, 4))).astype(BF)
    s = float(np.float32(scale).reshape(-1)[0])
    wqo = np.ascontiguousarray(wq, dtype=np.float32).astype(BF)
    wkt = np.ascontiguousarray(wk.T, dtype=np.float32).astype(BF)
    wvt = np.ascontiguousarray(wv.T * s, dtype=np.float32).astype(BF)
    bqb = np.ascontiguousarray(
        np.broadcast_to(np.float32(bq).reshape(C4, 1), (C4, K))).astype(BF)
    bvb = np.zeros((128, C), dtype=BF)
    bvb[0:K, :] = np.float32(bv).reshape(1, C) * s
    bkc = np.ascontiguousarray(bk, dtype=np.float32).reshape(C4, 1)
    pa = np.concatenate([wqo, bqb, bvb], axis=1)            # [128, 1056]
    return [
        {
            "x": x[i],
            "pa": pa,
            "pf": bkc,
            "pb": np.concatenate([wkt, yt[i], wvt], axis=1),
        }
        for i in range(N)
    ]


def _run(inputs, **kwargs):
    nc = _build()
    return run_bass_kernel_spmd(nc, _in_maps(**inputs),
                                core_ids=list(range(N)), **kwargs)


def kernel(**inputs) -> np.ndarray:
    res = _run(inputs)
    out = np.stack([np.asarray(res.results[i]["out"], dtype=np.float32)
                    for i in range(N)])
    return out.reshape(N, C, H, W)


# revision 16
# speedup vs baseline: 1.1742x; 1.0349x over previous
"""CPAMDec attention-decoder kernel for 8 Trainium2 NeuronCores.

Reference computation (per batch n of N=8):
    q  = x_n^T @ wq^T + bq          (HW=4096, C4=128)
    k  = y_n @ wk^T + bk            (K=32, C4=128)
    v  = y_n @ wv^T + bv            (K=32, C=512)
    attn = softmax(q @ k^T, axis=-1)        (HW, K)
    out = scale * (v^T @ attn^T) + x_n      (C, HW)

Sharding: pure data parallel - core i computes batch i; params replicated.

Key optimizations:
  - bf16 I/O. x and out move over HBM as bf16 (8MB -> 4MB each way per
    core); HBM-per-NC (~358 GB/s) is the binding roofline. rel-err of the
    full bf16 pipeline is ~3e-3, well under the 2e-2 gate.
  - wq folding: e[j,p] = sum_c EM[c,j] x[c,p] with EM = wq^T @ (k^T+bk)
    computed once in the prologue. The per-chunk q stage (4 matmuls + an
    ACT copy) disappears; energy comes straight from x.
  - bq contributes a per-key bias e_b[j] = sum_o bq[o]*ktb[o,j], applied
    inside the exp() activation (exact algebra).
  - scale folded into wv on host; s*bv baked into every v row in the
    prologue (sum_j attn = 1), so the per-chunk final is a plain
    o_ps + x tensor_tensor on DVE over double-bank PSUM tiles.
  - Consts ride in 3 packed DRAM params (DMA issue costs ~0.7us of queue
    time each; 10 separate loads would serialize startup by ~7us).
  - PE warm-up dummies ramp the HAM clock gate while DMAs land.
"""

import sys

sys.path.insert(0, "/opt/trn_rl_repo")

import numpy as np
import ml_dtypes

import concourse.bacc as bacc
import concourse.mybir as mybir
import concourse.tile as tile
from concourse.alu_op_type import AluOpType
from concourse.bass_utils import run_bass_kernel_spmd

F32 = mybir.dt.float32
BF16 = mybir.dt.bfloat16
AF = mybir.ActivationFunctionType
BF = ml_dtypes.bfloat16

N, C, H, W, K = 8, 512, 64, 64, 32
HW = H * W            # 4096
C4 = C // 4           # 128
PC = 512              # free-dim chunk (1 PSUM bank of fp32)
NPC = HW // PC        # 8 chunks
KC = C // 128         # 4 contraction chunks
CT = C // 128         # 4 output row-tiles


def _emit(nc, tc):
    sync = nc.sync
    cdma = nc.scalar      # consts ride the ACT HWDGE ring

    with (
        tc.tile_pool(name="const", bufs=1) as cst,
        tc.tile_pool(name="xbuf", bufs=1) as xp,
        tc.tile_pool(name="work", bufs=3) as wk_pool,
        tc.tile_pool(name="ps", bufs=2, space="PSUM") as ps,
    ):
        # ---------------- constant loads (3 packed DMAs) ----------------
        # pa = wqo[128,512] | bqb[128,32] | bvb[128,512]   bf16
        # pf = bk[128,1]                                   f32
        # pb = wkt | yt | wvt  as [512, 768] -> [128, 4, 768]  bf16
        pa = cst.tile([128, C + K + C], BF16, name="pa", tag="pa")
        cdma.dma_start(pa[:], nc.t.pa[:])
        wqo = pa[:, 0:C]
        bqb = pa[:, C:C + K]
        bvb32 = pa[0:K, C + K:C + K + C]
        pf = cst.tile([128, 1], F32, name="pf", tag="pf")
        cdma.dma_start(pf[:], nc.t.pf[:])
        bk_sb = pf[:, 0:1]
        pb = cst.tile([128, KC, 128 + 128 + C], BF16, name="pb", tag="pb")
        cdma.dma_start(pb[:], nc.t.pb[:].rearrange("(k p) f -> p k f", p=128))

        def wkt(k):
            return pb[:, k, 0:C4]

        def yt(k):
            return pb[:, k, 128:128 + 4 * K]

        def wvt(k):
            return pb[:, k, 256:256 + C]

        ones32 = cst.tile([K, 128], BF16, name="ones32", tag="ones32")
        nc.gpsimd.memset(ones32[:], 1.0)

        # x column chunks: (128 part, 4 c-tiles, PC cols) strided loads on
        # the SP ring. SBUF is plentiful: keep all 8 resident.
        xs = [None] * NPC

        def load_chunk(pc):
            t = xp.tile([128, KC, PC], BF16, name=f"xs{pc}", tag=f"xs{pc}")
            src = nc.t.x[:, pc * PC:(pc + 1) * PC].rearrange(
                "(k p) f -> p k f", p=128)
            sync.dma_start(t[:], src)
            xs[pc] = t

        for pc in range(4):
            load_chunk(pc)

        # ---------------- PE warm-up ----------------
        # Ramp the HAM clock gate (1.2 -> 2.4 GHz after ~3.4us sustained)
        # while DMAs land. Reads pa (first const to arrive).
        dmy_ps = ps.tile([128, PC], F32, name="dmy_ps", tag="s", bufs=1)
        for _ in range(7):
            nc.tensor.matmul(dmy_ps[:], pa[:, 0:128], wqo[:],
                             start=True, stop=True)

        # Load the exp ACT table before steady state (Copy/Identity live in
        # every table, so this is the only table load).
        acttbl = cst.tile([128, 8], BF16, name="acttbl", tag="acttbl")
        nc.scalar.activation(out=acttbl[:], in_=pa[:, 0:8], func=AF.Exp,
                             bias=0.0, scale=1.0)

        # ---------------- prologue ----------------
        # ktb[o,j] = sum_c wk[o,c] y[j,c] + bk[o]   (4K=128 j-replicas)
        kt_ps = ps.tile([C4, 4 * K], F32, name="kt_ps", tag="e", bufs=2)
        for k in range(KC):
            nc.tensor.matmul(kt_ps[:], wkt(k), yt(k),
                             start=(k == 0), stop=(k == KC - 1))
        ktb4 = cst.tile([C4, 4 * K], BF16, name="ktb4", tag="ktb4")
        nc.scalar.activation(out=ktb4[:], in_=kt_ps[:], func=AF.Identity,
                             bias=bk_sb, scale=1.0)

        # EM[c,j] = sum_o wq[o,c] ktb[o,j]  (c-tiled: [128, KC, 128])
        em_ps = ps.tile([128, KC, 128], F32, name="em_ps", tag="e", bufs=2)
        for k in range(KC):
            nc.tensor.matmul(em_ps[:, k, :], wqo[:, k * 128:(k + 1) * 128],
                             ktb4[:], start=True, stop=True)
        em_sb = cst.tile([128, KC, 128], BF16, name="em_sb", tag="em_sb")
        nc.scalar.activation(out=em_sb[:], in_=em_ps[:], func=AF.Copy,
                             bias=0.0, scale=1.0)

        # v[j,c] = sum_cl y[j,cl] wv(scaled)[c,cl], then += s*bv[c]
        v_ps = ps.tile([K, C], F32, name="v_ps", tag="s", bufs=1)
        for k in range(KC):
            nc.tensor.matmul(v_ps[:], yt(k)[:, 0:K], wvt(k),
                             start=(k == 0), stop=(k == KC - 1))
        v_sb = cst.tile([K, C], BF16, name="v_sb", tag="v_sb")
        nc.vector.tensor_tensor(v_sb[:], v_ps[:], bvb32,
                                op=AluOpType.add)
        # partition-stacked copy for row-packed final matmuls:
        # vstack[32*ct + j, m] = v_sb[j, 128*ct + m]
        vstack = cst.tile([128, 128], BF16, name="vstack", tag="vstack")
        for ct in range(CT):
            nc.gpsimd.dma_start(
                vstack[32 * ct:32 * (ct + 1), :],
                v_sb[:, 128 * ct:128 * (ct + 1)])

        # e_b[j] = sum_o bq[o] ktb[o,j] -> exp bias, per partition
        eb_ps = ps.tile([4 * K, K], F32, name="eb_ps", tag="o", bufs=2)
        nc.tensor.matmul(eb_ps[:], ktb4[:], bqb[:], start=True, stop=True)
        e_b4 = cst.tile([4 * K, 1], F32, name="e_b4", tag="e_b4")
        nc.scalar.activation(out=e_b4[:], in_=eb_ps[:, 0:1], func=AF.Copy,
                             bias=0.0, scale=1.0)

        # ------------- software-pipelined main loop over column chunks ----
        #   step i:  e/exp(i)   sum/rec/mul(i-1)   out-mm/add/store(i-2)
        expts = [None] * NPC
        attns = [None] * NPC

        def stage_e(pc):
            e_ps = ps.tile([128, PC], F32, name=f"e_ps{pc}", tag="e", bufs=2)
            for k in range(KC):
                nc.tensor.matmul(e_ps[:], em_sb[:, k, :], xs[pc][:, k, :],
                                 start=(k == 0), stop=(k == KC - 1))
            expt = wk_pool.tile([128, PC], BF16, name="expt", tag="expt",
                                bufs=3)
            nc.scalar.activation(out=expt[:], in_=e_ps[:], func=AF.Exp,
                                 bias=e_b4[:], scale=1.0)
            expts[pc] = expt

        def stage_s(pc):
            s_ps = ps.tile([128, PC], F32, name=f"s_ps{pc}", tag="s", bufs=1)
            nc.tensor.matmul(s_ps[:], ones32[:], expts[pc][0:K, :],
                             start=True, stop=True)
            rec = wk_pool.tile([128, PC], F32, name="rec", tag="rec", bufs=2)
            nc.vector.reciprocal_approx_fast(out=rec[:], in_=s_ps[:])
            attn = wk_pool.tile([128, PC], BF16, name="attn", tag="attn",
                                bufs=3)
            nc.gpsimd.tensor_tensor(attn[:], expts[pc][:], rec[:],
                                    op=AluOpType.mult)
            attns[pc] = attn

        def stage_out(pc):
            xt = xs[pc]
            attn = attns[pc]
            osb = wk_pool.tile([128, CT, PC], BF16, name="osb", tag="osb",
                               bufs=3)
            # two double-bank PSUM tiles; one 1024-col add per pair.
            # h0 detours through an ACT copy so its DVE add runs all-bf16
            # (2x_1p mode, ~half cost); h1 adds straight from PSUM.
            for h in range(2):
                o_ps = ps.tile([128, 2, PC], F32, name=f"o_ps{pc}_{h}",
                               tag="o", bufs=2)
                for i in range(2):
                    ct = 2 * h + i
                    nc.tensor.matmul(o_ps[:, i, :],
                                     vstack[32 * ct:32 * (ct + 1), :],
                                     attn[32 * ct:32 * (ct + 1), :],
                                     start=True, stop=True,
                                     tile_position=(32 * ct, 0))
                if h == 0:
                    tmp = wk_pool.tile([128, 2, PC], BF16, name="tmp",
                                       tag="tmp", bufs=2)
                    nc.scalar.activation(out=tmp[:], in_=o_ps[:],
                                         func=AF.Copy, bias=0.0, scale=1.0)
                    nc.vector.tensor_tensor(osb[:, 0:2, :], tmp[:],
                                            xt[:, 0:2, :], op=AluOpType.add)
                else:
                    nc.vector.tensor_tensor(osb[:, 2:4, :], o_ps[:],
                                            xt[:, 2:4, :], op=AluOpType.add)
                dst = nc.t.out[2 * h * 128:(2 * h + 2) * 128,
                               pc * PC:(pc + 1) * PC].rearrange(
                    "(k p) f -> p k f", p=128)
                sync.dma_start(dst, osb[:, 2 * h:2 * h + 2, :])

        for step in range(NPC + 3):
            if 1 <= step and step + 3 < NPC:
                load_chunk(step + 3)
            if step < NPC:
                stage_e(step)
            if 0 <= step - 1 < NPC:
                stage_s(step - 1)
            if 0 <= step - 2 < NPC:
                stage_out(step - 2)


class _T:
    """Attribute access to declared dram params."""
    def __init__(self):
        self.__dict__ = {}


_NC_CACHE = []


def _build():
    if _NC_CACHE:
        return _NC_CACHE[0]
    nc = bacc.Bacc(target_bir_lowering=False)
    nc.t = _T()
    t = nc.t
    t.x = nc.declare_dram_parameter("x", [C, HW], BF16, isOutput=False)
    t.pa = nc.declare_dram_parameter("pa", [128, C + K + C], BF16,
                                     isOutput=False)
    t.pf = nc.declare_dram_parameter("pf", [128, 1], F32, isOutput=False)
    t.pb = nc.declare_dram_parameter("pb", [C, 256 + C], BF16, isOutput=False)
    t.out = nc.declare_dram_parameter("out", [C, HW], BF16, isOutput=True)
    with tile.TileContext(nc) as tc:
        _emit(nc, tc)
    nc.finalize()
    _NC_CACHE.append(nc)
    return nc


def _in_maps(x, y, wq, bq, wk, bk, wv, bv, scale):
    x = np.ascontiguousarray(x, dtype=np.float32).reshape(N, C, HW).astype(BF)
    yt = np.ascontiguousarray(
        np.tile(np.transpose(y, (0, 2, 1)), (1, 1, 4))).astype(BF)
    s = float(np.float32(scale).reshape(-1)[0])
    wqo = np.ascontiguousarray(wq, dtype=np.float32).astype(BF)
    wkt = np.ascontiguousarray(wk.T, dtype=np.float32).astype(BF)
    wvt = np.ascontiguousarray(wv.T * s, dtype=np.float32).astype(BF)
    bqb = np.ascontiguousarray(
        np.broadcast_to(np.float32(bq).reshape(C4, 1), (C4, K))).astype(BF)
    bvb = np.zeros((128, C), dtype=BF)
    bvb[0:K, :] = np.float32(bv).reshape(1, C) * s
    bkc = np.ascontiguousarray(bk, dtype=np.float32).reshape(C4, 1)
    pa = np.concatenate([wqo, bqb, bvb], axis=1)            # [128, 1056]
    return [
        {
            "x": x[i],
            "pa": pa,
            "pf": bkc,
            "pb": np.concatenate([wkt, yt[i], wvt], axis=1),
        }
        for i in range(N)
    ]


def _run(inputs, **kwargs):
    nc = _build()
    return run_bass_kernel_spmd(nc, _in_maps(**inputs),
                                core_ids=list(range(N)), **kwargs)


def kernel(**inputs) -> np.ndarray:
    res = _run(inputs)
    out = np.stack([np.asarray(res.results[i]["out"], dtype=np.float32)
                    for i in range(N)])
    return out.reshape(N, C, H, W)
